# revision 6
# baseline (speedup 1.0000x reference)
"""Chebyshev-KAN (2 layers) Trainium2 kernel, 8-core SPMD.

Math (per layer): per-feature min/max normalize x over the batch to [-1,1],
Chebyshev expansion T_0..T_8, contract: out[b,o] = sum_{i,d} T_d(xn[b,i]) *
coeffs[i,o,d].

Distribution: 2-way data-parallel over batch (bh) x 4-way tensor-parallel
over the layer output dim (og); core = bh*4 + og.
  - Layer-1 per-feature min/max: each core scans 1/8 of the batch,
    one AllReduce(max) over [max | -min] merges all cores.
  - Layer-1 outputs (h) are produced feature-major [o1_local, b_local]; a
    chunked AllGather (one per 512-batch block, over the 4-core og group)
    rebuilds the full 2048 features each core needs for layer 2.
  - Layer-2 min/max: local per-feature stats + small AllGather + pairwise
    max merge across the two batch halves.

Compute strategy: the Chebyshev basis is built in a bounded "product basis"
  B1=W1, B2=W1^2, B3=W2*W1, B4=W2^2, B5=W2*B3, B6=B3^2, B7=W4*B3, B8=W4^2
(W_d = 2*T_d(xn); every |B_k| <= 16), which costs only 1 tensor_scalar +
3 two-input DVE ops + 4 ScalarE squares per 128x512 chunk. Coefficients are
re-expressed in this basis on the host (exact, linear, small integers); the
d=0 term and basis constants fold into a per-output bias added on PSUM
copy-out. Matmuls run in fp32r (TF32-like, 11-bit mantissa, full PE rate),
accumulating fp32 in PSUM.
"""

import sys

import numpy as np

try:
    import concourse  # noqa: F401
except ImportError:  # pragma: no cover
    sys.path.insert(0, "/opt/trn_rl_repo")

import concourse.tile as tile  # noqa: E402
from concourse import bacc, mybir  # noqa: E402
from concourse.bass_utils import run_bass_kernel_spmd  # noqa: E402

F32 = mybir.dt.float32
F32R = mybir.dt.float32r
FP16 = mybir.dt.float16
FP16NP = mybir.dt.np(mybir.dt.float16)
ALU = mybir.AluOpType
ACTF = mybir.ActivationFunctionType
AX = mybir.AxisListType

N_CORES = 8
P_B, P_O = 2, 4            # batch halves x output-dim quarters
BATCH, IN1, HID, OUT = 16384, 1024, 2048, 1024
BBLK = 512                 # batch block (one PSUM bank of fp32)


# --------------------------------------------------------------------------
# host-side math helpers
# --------------------------------------------------------------------------

def _round_fp32r(a: np.ndarray) -> np.ndarray:
    """Round fp32 to fp32r (1s+8e+11m; low 12 bits zero), nearest-even."""
    u = a.astype(np.float32).view(np.uint32)
    lo = u & np.uint32(0xFFF)
    hi = u >> np.uint32(12)
    add = (lo > 0x800) | ((lo == 0x800) & ((hi & 1) == 1))
    return ((hi + add.astype(np.uint32)) << np.uint32(12)).view(np.float32)


def _to_product_basis(c: np.ndarray):
    """c [i, o, 9] (T basis) -> p [i, o, 8] (product basis) + kappa [i, o].

    sum_d c_d T_d == sum_k p_k B_k + kappa  (exact, in float64).
    """
    c = c.astype(np.float64)
    c0, c1, c2, c3, c4, c5, c6, c7, c8 = [c[..., d] for d in range(9)]
    p = np.empty(c.shape[:-1] + (8,), np.float64)
    p[..., 0] = 0.5 * (c1 - c3 - c5 + c7)   # B1
    p[..., 1] = 0.5 * (c2 - 3.0 * c6)       # B2
    p[..., 2] = 0.5 * (c3 - c5)             # B3
    p[..., 3] = 0.5 * (c4 - 2.0 * c6)       # B4
    p[..., 4] = 0.5 * (c5 - c7)             # B5
    p[..., 5] = 0.5 * c6                    # B6
    p[..., 6] = 0.5 * c7                    # B7
    p[..., 7] = 0.5 * c8                    # B8
    kappa = c0 - c2 - c4 + 3.0 * c6 - c8
    return p, kappa


def basis_values(xn: np.ndarray) -> list:
    """Reference values of B_1..B_8 for normalized input (for testing)."""
    w1 = 2.0 * xn
    b1 = w1
    b2 = w1 * w1                 # W2 + 2
    b3 = (b2 - 2.0) * b1         # W3 + W1
    b4 = (b2 - 2.0) ** 2         # W4 + 2
    b5 = (b2 - 2.0) * b3         # W5 + W3 + 2W1
    b6 = b3 * b3                 # W6 + 2W4 + 3W2 + 4
    b7 = (b4 - 2.0) * b3         # W7 + W5 + W3 + W1
    b8 = (b4 - 2.0) ** 2         # W8 + 2
    return [b1, b2, b3, b4, b5, b6, b7, b8]


def _pack_weights(p: np.ndarray, n_ic: int, n_oc: int) -> np.ndarray:
    """p [I, O_local, 8] -> [128, n_ic*8*n_oc*128] fp32 (fp32r-rounded).

    SBUF layout: partition = i within 128-chunk; free index =
    (((ic*8 + k)*n_oc + oc)*128 + o).
    """
    I, OL, K = p.shape
    assert I == n_ic * 128 and OL == n_oc * 128 and K == 8
    a = p.reshape(n_ic, 128, n_oc, 128, 8)          # (ic, i, oc, o, k)
    a = a.transpose(1, 0, 4, 2, 3)                  # (i, ic, k, oc, o)
    return np.ascontiguousarray(a).reshape(128, -1).astype(
        np.float32).astype(FP16NP)


def _pack_bias(bias: np.ndarray) -> np.ndarray:
    """bias [O_local] -> [128, n_oc] (partition = o within chunk)."""
    n_oc = bias.shape[0] // 128
    return np.ascontiguousarray(bias.reshape(n_oc, 128).T).astype(np.float32)


# --------------------------------------------------------------------------
# device program
# --------------------------------------------------------------------------

def _emit_basis(nc, bpool, src_ap, a_ap, c_ap, n, neg2):
    """Expand one [128, n] chunk into the 8 basis tiles (fp32r), returned as
    one [128, 8*n] tile. B1 = a*x + c (per-partition affine) = 2*xn."""
    basis = bpool.tile([128, 8 * n], FP16, tag="basis")

    def b(k):
        return basis[:, (k - 1) * n:k * n]

    sq = ACTF.Square
    nc.vector.tensor_scalar(b(1), src_ap, a_ap, c_ap, ALU.mult, ALU.add)
    nc.scalar.activation(b(2), b(1), sq)
    nc.vector.scalar_tensor_tensor(b(3), b(2), -2.0, b(1), ALU.add, ALU.mult)
    nc.scalar.activation(b(4), b(2), sq, bias=neg2)
    nc.vector.scalar_tensor_tensor(b(5), b(2), -2.0, b(3), ALU.add, ALU.mult)
    nc.scalar.activation(b(6), b(3), sq)
    nc.vector.scalar_tensor_tensor(b(7), b(4), -2.0, b(3), ALU.add, ALU.mult)
    nc.scalar.activation(b(8), b(4), sq, bias=neg2)
    return basis


def _emit_norm_scalars(nc, pool, mx_ap, negmn_ap, out_a, out_c, n):
    """From per-feature max and -min ([128, n]) compute a = 4/rng and
    c = -2*(mx+mn)/rng, so that a*x + c = 2*xn."""
    rng = pool.tile([128, n], F32, tag="normtmp")
    nc.vector.tensor_tensor(rng[:], mx_ap, negmn_ap, ALU.add)        # mx - mn
    rcp = pool.tile([128, n], F32, tag="normtmp")
    nc.vector.reciprocal(rcp[:], rng[:])
    nc.vector.tensor_scalar(out_a, rcp[:], 4.0, None, ALU.mult)      # 4/rng
    s1 = pool.tile([128, n], F32, tag="normtmp")
    nc.vector.tensor_tensor(s1[:], mx_ap, negmn_ap, ALU.subtract)    # mx + mn
    nc.vector.tensor_tensor(s1[:], s1[:], rcp[:], ALU.mult)
    nc.vector.tensor_scalar(out_c, s1[:], -2.0, None, ALU.mult)


def build_bass(batch=BATCH, compile_=True, repeat=1, comm="full"):
    """Build the SPMD-8 Bass program. `batch` can be scaled down for sim."""
    b_local = batch // P_B
    n_blk = b_local // BBLK
    n_ic1, n_oc1 = IN1 // 128, HID // P_O // 128      # 8, 4
    n_ic2, n_oc2 = HID // 128, OUT // P_O // 128      # 16, 2
    bs_local = batch // N_CORES                       # stats slice per core

    nc = bacc.Bacc("TRN2", target_bir_lowering=False, debug=False,
                   num_devices=N_CORES)

    xh = nc.dram_tensor("xh", [IN1, b_local], F32, kind="ExternalInput").ap()
    xs = nc.dram_tensor("xs", [IN1, bs_local], F32, kind="ExternalInput").ap()
    w1 = nc.dram_tensor("w1", [128, n_ic1 * 8 * n_oc1 * 128], FP16,
                        kind="ExternalInput").ap()
    w2 = nc.dram_tensor("w2", [128, n_ic2 * 8 * n_oc2 * 128], FP16,
                        kind="ExternalInput").ap()
    bias1_d = nc.dram_tensor("bias1", [128, n_oc1], F32,
                             kind="ExternalInput").ap()
    bias2_d = nc.dram_tensor("bias2", [128, n_oc2], F32,
                             kind="ExternalInput").ap()
    y = nc.dram_tensor("y", [n_oc2 * 128, b_local], F32,
                       kind="ExternalOutput").ap()

    grp_og = [[bh * P_O + og for og in range(P_O)] for bh in range(P_B)]
    grp_all = [list(range(N_CORES))]

    with tile.TileContext(nc) as tc:
        with (
            tc.tile_pool(name="w", bufs=1) as wpool,
            tc.tile_pool(name="basis", bufs=2) as bpool,
            tc.tile_pool(name="xin", bufs=4) as xpool,
            tc.tile_pool(name="xstat", bufs=1) as xspool,
            tc.tile_pool(name="hst", bufs=4) as hpool,
            tc.tile_pool(name="small", bufs=1) as spool,
            tc.tile_pool(name="stmp", bufs=4) as stpool,
            tc.tile_pool(name="acc", bufs=8, space="PSUM") as ppool,
            tc.tile_pool(name="dstat", bufs=4, space="DRAM") as dspool,
            tc.tile_pool(name="dhin", bufs=max(n_blk, 3), space="DRAM") as dhin,
            tc.tile_pool(name="dhout", bufs=max(n_blk, 1), space="DRAM") as dhout,
        ):
            for _rep in range(repeat):
                # ---- biases ----
                neg2t = spool.tile([128, 1], F32)
                nc.vector.memset(neg2t[:], -2.0)
                b1sb = spool.tile([128, n_oc1], F32)
                nc.sync.dma_start(b1sb[:], bias1_d[:])
                b2sb = spool.tile([128, n_oc2], F32)
                nc.sync.dma_start(b2sb[:], bias2_d[:])

                # ======== L1 stats: per-feature min/max of x, all-reduced ======
                st1 = spool.tile([128, 2 * n_ic1], F32)
                for ic in range(n_ic1):
                    t = xspool.tile([128, bs_local], F32, tag="xstats")
                    nc.sync.dma_start(t[:], xs[ic * 128:(ic + 1) * 128, :])
                    nc.vector.tensor_reduce(st1[:, ic:ic + 1], t[:], AX.X, ALU.max)
                    mn = stpool.tile([128, 1], F32, tag="mn")
                    nc.vector.tensor_reduce(mn[:], t[:], AX.X, ALU.min)
                    nc.vector.tensor_scalar(st1[:, n_ic1 + ic:n_ic1 + ic + 1],
                                            mn[:], -1.0, None, ALU.mult)
                if comm != "none":
                    ar_in = dspool.tile([128, 2 * n_ic1], F32, tag="dstat")
                    ar_out = dspool.tile([128, 2 * n_ic1], F32, tag="dstat")
                    nc.sync.dma_start(ar_in[:], st1[:])
                    nc.gpsimd.collective_compute(
                        "AllReduce", ALU.max, replica_groups=grp_all,
                        ins=[ar_in.opt()], outs=[ar_out.opt()])
                    stg = spool.tile([128, 2 * n_ic1], F32)
                    nc.sync.dma_start(stg[:], ar_out[:])
                else:
                    stg = st1
                a1 = spool.tile([128, n_ic1], F32)
                c1 = spool.tile([128, n_ic1], F32)
                _emit_norm_scalars(nc, stpool, stg[:, 0:n_ic1], stg[:, n_ic1:],
                                   a1[:], c1[:], n_ic1)

                # ---- L1 weights ----
                wsb = wpool.tile([128, n_ic1 * 8 * n_oc1 * 128], FP16, tag="w")
                seg = 8 * n_oc1 * 128
                for ic in range(n_ic1):
                    nc.sync.dma_start(wsb[:, ic * seg:(ic + 1) * seg],
                                      w1[:, ic * seg:(ic + 1) * seg])

                # ---- L1 main loop ----
                hmax = spool.tile([128, n_oc1], F32)
                hmin = spool.tile([128, n_oc1], F32)
                hag_outs = []
                hag_ins = []
                if comm == "batched":
                    hag_in_full = dhin.tile([n_oc1 * 128, b_local], FP16,
                                            tag="hag_in_full")
                    hag_out_full = dhout.tile([HID, b_local], FP16,
                                              tag="hag_out_full")
                for blk in range(n_blk):
                    ps = [ppool.tile([128, BBLK], F32, tag="acc",
                                     name=f"ps1_{_rep}_{blk}_{i}")
                          for i in range(n_oc1)]
                    for ic in range(n_ic1):
                        xt = xpool.tile([128, BBLK], F32, tag="x")
                        nc.sync.dma_start(
                            xt[:], xh[ic * 128:(ic + 1) * 128,
                                      blk * BBLK:(blk + 1) * BBLK])
                        basis = _emit_basis(nc, bpool, xt[:],
                                            a1[:, ic:ic + 1], c1[:, ic:ic + 1],
                                            BBLK, neg2t[:])
                        for k in range(8):
                            rhs = basis[:, k * BBLK:(k + 1) * BBLK]
                            for oc in range(n_oc1):
                                off = (((ic * 8 + k) * n_oc1 + oc) * 128)
                                nc.tensor.matmul(
                                    ps[oc][:], wsb[:, off:off + 128], rhs,
                                    start=(ic == 0 and k == 0),
                                    stop=(ic == n_ic1 - 1 and k == 7))
                    # epilogue: bias add, running h stats, stage + AllGather
                    if comm == "batched":
                        hag_in = hag_in_full
                        hag_out = hag_out_full
                    else:
                        hag_in = dhin.tile([n_oc1 * 128, BBLK], FP16,
                                           tag="hag_in")
                        hag_out = dhout.tile([HID, BBLK], FP16, tag="hag_out")
                    hag_outs.append(hag_out)
                    hag_ins.append(hag_in)
                    for oc in range(n_oc1):
                        hsb = hpool.tile([128, BBLK], FP16, tag="h")
                        nc.scalar.activation(hsb[:], ps[oc][:], ACTF.Identity,
                                             bias=b1sb[:, oc:oc + 1])
                        if blk == 0:
                            nc.vector.tensor_reduce(hmax[:, oc:oc + 1], hsb[:],
                                                    AX.X, ALU.max)
                            nc.vector.tensor_reduce(hmin[:, oc:oc + 1], hsb[:],
                                                    AX.X, ALU.min)
                        else:
                            tmx = stpool.tile([128, 1], F32, tag="tmx")
                            nc.vector.tensor_reduce(tmx[:], hsb[:], AX.X, ALU.max)
                            nc.vector.tensor_tensor(hmax[:, oc:oc + 1],
                                                    hmax[:, oc:oc + 1], tmx[:],
                                                    ALU.max)
                            tmn = stpool.tile([128, 1], F32, tag="tmn")
                            nc.vector.tensor_reduce(tmn[:], hsb[:], AX.X, ALU.min)
                            nc.vector.tensor_tensor(hmin[:, oc:oc + 1],
                                                    hmin[:, oc:oc + 1], tmn[:],
                                                    ALU.min)
                        if comm == "batched":
                            nc.sync.dma_start(
                                hag_in[oc * 128:(oc + 1) * 128,
                                       blk * BBLK:(blk + 1) * BBLK], hsb[:])
                        else:
                            nc.sync.dma_start(
                                hag_in[oc * 128:(oc + 1) * 128, :], hsb[:])
                    if comm == "full":
                        nc.gpsimd.collective_compute(
                            "AllGather", ALU.bypass, replica_groups=grp_og,
                            ins=[hag_in.opt()], outs=[hag_out.opt()])
                if comm == "batched":
                    nc.gpsimd.collective_compute(
                        "AllGather", ALU.bypass, replica_groups=grp_og,
                        ins=[hag_in_full.opt()], outs=[hag_out_full.opt()])

                # ======== L2 stats ========
                st2 = spool.tile([128, 2 * n_oc1], F32)
                nc.vector.tensor_copy(st2[:, 0:n_oc1], hmax[:])
                nc.vector.tensor_scalar(st2[:, n_oc1:], hmin[:], -1.0, None,
                                        ALU.mult)
                ag2_in = dspool.tile([128, 2 * n_oc1], F32, tag="dstat")
                ag2_out = dspool.tile([128 * N_CORES, 2 * n_oc1], F32,
                                      tag="dstat2")
                nc.sync.dma_start(ag2_in[:], st2[:])
                if comm != "none":
                    nc.gpsimd.collective_compute(
                        "AllGather", ALU.bypass, replica_groups=grp_all,
                        ins=[ag2_in.opt()], outs=[ag2_out.opt()])
                else:
                    nc.sync.dma_start(ag2_out[0:128, :], ag2_in[:])
                g2 = spool.tile([128, N_CORES * 2 * n_oc1], F32)
                nc.sync.dma_start(
                    g2[:], ag2_out[:].rearrange("(r p) c -> p r c", p=128))
                half = P_O * 2 * n_oc1
                m2 = spool.tile([128, half], F32)
                nc.vector.tensor_tensor(m2[:], g2[:, 0:half], g2[:, half:],
                                        ALU.max)
                # compact the strided og-blocked layout into [128, n_ic2]
                mxc = spool.tile([128, n_ic2], F32)
                nmc = spool.tile([128, n_ic2], F32)
                m2v = m2[:].rearrange("p (og c) -> p og c", og=P_O)
                nc.vector.tensor_copy(
                    mxc[:].rearrange("p (og c) -> p og c", og=P_O),
                    m2v[:, :, 0:n_oc1])
                nc.vector.tensor_copy(
                    nmc[:].rearrange("p (og c) -> p og c", og=P_O),
                    m2v[:, :, n_oc1:2 * n_oc1])
                a2 = spool.tile([128, n_ic2], F32)
                c2 = spool.tile([128, n_ic2], F32)
                _emit_norm_scalars(nc, stpool, mxc[:], nmc[:], a2[:], c2[:],
                                   n_ic2)

                # ---- L2 weights (reuses the w slot; waits for last w1 use) ----
                wsb2 = wpool.tile([128, n_ic2 * 8 * n_oc2 * 128], FP16, tag="w")
                seg2 = 8 * n_oc2 * 128
                for ic in range(n_ic2):
                    nc.sync.dma_start(wsb2[:, ic * seg2:(ic + 1) * seg2],
                                      w2[:, ic * seg2:(ic + 1) * seg2])

                # ---- L2 main loop ----
                for blk in range(n_blk):
                    ps = [ppool.tile([128, BBLK], F32, tag="acc",
                                     name=f"ps2_{blk}_{i}")
                          for i in range(n_oc2)]
                    for ic in range(n_ic2):
                        ht = xpool.tile([128, BBLK], FP16, tag="x")
                        src_hag = hag_outs[blk]
                        src_row = ic * 128 if comm != "none" else (ic % n_oc1) * 128
                        if comm == "none":
                            src_hag = hag_ins[blk]
                        if comm == "batched":
                            nc.sync.dma_start(
                                ht[:], src_hag[src_row:src_row + 128,
                                               blk * BBLK:(blk + 1) * BBLK])
                        else:
                            nc.sync.dma_start(
                                ht[:], src_hag[src_row:src_row + 128, :])
                        basis = _emit_basis(nc, bpool, ht[:],
                                            a2[:, ic:ic + 1], c2[:, ic:ic + 1],
                                            BBLK, neg2t[:])
                        for k in range(8):
                            rhs = basis[:, k * BBLK:(k + 1) * BBLK]
                            for oc in range(n_oc2):
                                off = (((ic * 8 + k) * n_oc2 + oc) * 128)
                                nc.tensor.matmul(
                                    ps[oc][:], wsb2[:, off:off + 128], rhs,
                                    start=(ic == 0 and k == 0),
                                    stop=(ic == n_ic2 - 1 and k == 7))
                    for oc in range(n_oc2):
                        ysb = hpool.tile([128, BBLK], F32, tag="h")
                        nc.scalar.activation(ysb[:], ps[oc][:], ACTF.Identity,
                                             bias=b2sb[:, oc:oc + 1])
                        nc.sync.dma_start(
                            y[oc * 128:(oc + 1) * 128,
                              blk * BBLK:(blk + 1) * BBLK], ysb[:])

    if compile_:
        nc.compile()
    return nc


# --------------------------------------------------------------------------
# host wrapper
# --------------------------------------------------------------------------

def prepare_in_maps(x, coeffs1, coeffs2, batch=BATCH):
    xT = np.ascontiguousarray(x.T)                       # [IN1, batch]
    b_local = batch // P_B
    bs_local = batch // N_CORES

    p1, k1 = _to_product_basis(coeffs1)                  # [IN1, HID, 8]
    p2, k2 = _to_product_basis(coeffs2)
    bias1 = k1.sum(axis=0)                               # [HID]
    bias2 = k2.sum(axis=0)                               # [OUT]

    o1, o2 = HID // P_O, OUT // P_O
    in_maps = []
    packs = []
    for og in range(P_O):
        packs.append((
            _pack_weights(p1[:, og * o1:(og + 1) * o1, :], IN1 // 128,
                          o1 // 128),
            _pack_bias(bias1[og * o1:(og + 1) * o1]),
            _pack_weights(p2[:, og * o2:(og + 1) * o2, :], HID // 128,
                          o2 // 128),
            _pack_bias(bias2[og * o2:(og + 1) * o2]),
        ))
    for core in range(N_CORES):
        bh, og = divmod(core, P_O)
        w1p, b1p, w2p, b2p = packs[og]
        in_maps.append(dict(
            xh=np.ascontiguousarray(xT[:, bh * b_local:(bh + 1) * b_local]),
            xs=np.ascontiguousarray(
                xT[:, core * bs_local:(core + 1) * bs_local]),
            w1=w1p, w2=w2p, bias1=b1p, bias2=b2p,
        ))
    return in_maps


def assemble_output(results, batch=BATCH):
    b_local = batch // P_B
    o2 = OUT // P_O
    y = np.empty((batch, OUT), np.float32)
    for core in range(N_CORES):
        bh, og = divmod(core, P_O)
        y[bh * b_local:(bh + 1) * b_local, og * o2:(og + 1) * o2] = \
            results[core]["y"].T
    return y


def _disable_birsim():
    """Skip walrus's BIR-simulator pass (compile-time only; big speedup)."""
    import concourse.bass_utils as bu
    if getattr(bu, "_birsim_patched", False):
        return
    orig = bu.run_command

    def patched(cmd, **kw):
        cmd = [c.replace("--enable-birsim=true", "--enable-birsim=false")
               if isinstance(c, str) else c for c in cmd]
        return orig(cmd, **kw)

    bu.run_command = patched
    bu._birsim_patched = True


_NC_CACHE = {}


def kernel(x, coeffs1, coeffs2):
    assert x.shape == (BATCH, IN1), x.shape
    _disable_birsim()
    if "nc" not in _NC_CACHE:
        _NC_CACHE["nc"] = build_bass(BATCH, comm="full")
    nc = _NC_CACHE["nc"]
    in_maps = prepare_in_maps(np.asarray(x, np.float32),
                              np.asarray(coeffs1, np.float32),
                              np.asarray(coeffs2, np.float32), BATCH)
    res = run_bass_kernel_spmd(nc, in_maps, core_ids=list(range(N_CORES)))
    return assemble_output(res.results, BATCH)



# revision 14
# speedup vs baseline: 1.6120x; 1.6120x over previous
"""Chebyshev-KAN (2 layers) Trainium2 kernel, 8-core SPMD.

Math (per layer): per-feature min/max normalize x over the batch to [-1,1],
Chebyshev expansion T_0..T_8, contract: out[b,o] = sum_{i,d} T_d(xn[b,i]) *
coeffs[i,o,d].

Distribution: 2-way data-parallel over batch (bh) x 4-way tensor-parallel
over the layer output dim (og); core = bh*4 + og.
  - Layer-1 per-feature min/max: each core scans 1/8 of the batch,
    one AllReduce(max) over [max | -min] merges all cores.
  - Layer-1 outputs (h) are produced feature-major [o1_local, b_local]; a
    chunked AllGather (one per 512-batch block, over the 4-core og group)
    rebuilds the full 2048 features each core needs for layer 2.
  - Layer-2 min/max: local per-feature stats + small AllGather + pairwise
    max merge across the two batch halves.

Compute strategy: the Chebyshev basis is built in a bounded "product basis"
  B1=W1, B2=W1^2, B3=W2*W1, B4=W2^2, B5=W2*B3, B6=B3^2, B7=W4*B3, B8=W4^2
(W_d = 2*T_d(xn); every |B_k| <= 16), which costs only 1 tensor_scalar +
3 two-input DVE ops + 4 ScalarE squares per 128x512 chunk. Coefficients are
re-expressed in this basis on the host (exact, linear, small integers); the
d=0 term and basis constants fold into a per-output bias added on PSUM
copy-out. Matmuls run in fp32r (TF32-like, 11-bit mantissa, full PE rate),
accumulating fp32 in PSUM.
"""

import sys

import numpy as np

try:
    import concourse  # noqa: F401
except ImportError:  # pragma: no cover
    sys.path.insert(0, "/opt/trn_rl_repo")

import concourse.tile as tile  # noqa: E402
from concourse import bacc, mybir  # noqa: E402
from concourse.bass_utils import run_bass_kernel_spmd  # noqa: E402

F32 = mybir.dt.float32
F32R = mybir.dt.float32r
FP16 = mybir.dt.float16
FP16NP = mybir.dt.np(mybir.dt.float16)
ALU = mybir.AluOpType
ACTF = mybir.ActivationFunctionType
AX = mybir.AxisListType

N_CORES = 8
P_B, P_O = 2, 4            # batch halves x output-dim quarters
BATCH, IN1, HID, OUT = 16384, 1024, 2048, 1024
BBLK = 512                 # batch block (one PSUM bank of fp32)


# --------------------------------------------------------------------------
# host-side math helpers
# --------------------------------------------------------------------------

def _round_fp32r(a: np.ndarray) -> np.ndarray:
    """Round fp32 to fp32r (1s+8e+11m; low 12 bits zero), nearest-even."""
    u = a.astype(np.float32).view(np.uint32)
    lo = u & np.uint32(0xFFF)
    hi = u >> np.uint32(12)
    add = (lo > 0x800) | ((lo == 0x800) & ((hi & 1) == 1))
    return ((hi + add.astype(np.uint32)) << np.uint32(12)).view(np.float32)


def _to_product_basis(c: np.ndarray):
    """c [i, o, 9] (T basis) -> p [i, o, 8] (product basis) + kappa [i, o].

    sum_d c_d T_d == sum_k p_k B_k + kappa  (exact, in float64).
    """
    c = c.astype(np.float64)
    c0, c1, c2, c3, c4, c5, c6, c7, c8 = [c[..., d] for d in range(9)]
    p = np.empty(c.shape[:-1] + (8,), np.float64)
    p[..., 0] = 0.5 * (c1 - c3 - c5 + c7)   # B1
    p[..., 1] = 0.5 * (c2 - 3.0 * c6)       # B2
    p[..., 2] = 0.5 * (c3 - c5)             # B3
    p[..., 3] = 0.5 * (c4 - 2.0 * c6)       # B4
    p[..., 4] = 0.5 * (c5 - c7)             # B5
    p[..., 5] = 0.5 * c6                    # B6
    p[..., 6] = 0.5 * c7                    # B7
    p[..., 7] = 0.5 * c8                    # B8
    kappa = c0 - c2 - c4 + 3.0 * c6 - c8
    return p, kappa


def basis_values(xn: np.ndarray) -> list:
    """Reference values of B_1..B_8 for normalized input (for testing)."""
    w1 = 2.0 * xn
    b1 = w1
    b2 = w1 * w1                 # W2 + 2
    b3 = (b2 - 2.0) * b1         # W3 + W1
    b4 = (b2 - 2.0) ** 2         # W4 + 2
    b5 = (b2 - 2.0) * b3         # W5 + W3 + 2W1
    b6 = b3 * b3                 # W6 + 2W4 + 3W2 + 4
    b7 = (b4 - 2.0) * b3         # W7 + W5 + W3 + W1
    b8 = (b4 - 2.0) ** 2         # W8 + 2
    return [b1, b2, b3, b4, b5, b6, b7, b8]


def _pack_weights(p: np.ndarray, n_ic: int, n_oc: int) -> np.ndarray:
    """p [I, O_local, 8] -> [128, n_ic*8*n_oc*128] fp32 (fp32r-rounded).

    SBUF layout: partition = i within 128-chunk; free index =
    (((ic*8 + k)*n_oc + oc)*128 + o).
    """
    I, OL, K = p.shape
    assert I == n_ic * 128 and OL == n_oc * 128 and K == 8
    a = p.reshape(n_ic, 128, n_oc, 128, 8)          # (ic, i, oc, o, k)
    a = a.transpose(1, 0, 4, 2, 3)                  # (i, ic, k, oc, o)
    return np.ascontiguousarray(a).reshape(128, -1).astype(
        np.float32).astype(FP16NP)


def _pack_bias(bias: np.ndarray) -> np.ndarray:
    """bias [O_local] -> [128, n_oc] (partition = o within chunk)."""
    n_oc = bias.shape[0] // 128
    return np.ascontiguousarray(bias.reshape(n_oc, 128).T).astype(np.float32)


# --------------------------------------------------------------------------
# device program
# --------------------------------------------------------------------------

def _emit_basis(nc, bpool, src_ap, a_ap, c_ap, n, neg2):
    """Expand one [128, n] chunk into the 8 basis tiles (fp32r), returned as
    one [128, 8*n] tile. B1 = a*x + c (per-partition affine) = 2*xn."""
    basis = bpool.tile([128, 8 * n], FP16, tag="basis")

    def b(k):
        return basis[:, (k - 1) * n:k * n]

    sq = ACTF.Square
    nc.vector.tensor_scalar(b(1), src_ap, a_ap, c_ap, ALU.mult, ALU.add)
    nc.scalar.activation(b(2), b(1), sq)
    nc.vector.scalar_tensor_tensor(b(3), b(2), -2.0, b(1), ALU.add, ALU.mult)
    nc.scalar.activation(b(4), b(2), sq, bias=neg2)
    nc.vector.scalar_tensor_tensor(b(5), b(2), -2.0, b(3), ALU.add, ALU.mult)
    nc.scalar.activation(b(6), b(3), sq)
    nc.vector.scalar_tensor_tensor(b(7), b(4), -2.0, b(3), ALU.add, ALU.mult)
    nc.scalar.activation(b(8), b(4), sq, bias=neg2)
    return basis


def _emit_norm_scalars(nc, pool, mx_ap, negmn_ap, out_a, out_c, n):
    """From per-feature max and -min ([128, n]) compute a = 4/rng and
    c = -2*(mx+mn)/rng, so that a*x + c = 2*xn."""
    rng = pool.tile([128, n], F32, tag="normtmp")
    nc.vector.tensor_tensor(rng[:], mx_ap, negmn_ap, ALU.add)        # mx - mn
    rcp = pool.tile([128, n], F32, tag="normtmp")
    nc.vector.reciprocal(rcp[:], rng[:])
    nc.vector.tensor_scalar(out_a, rcp[:], 4.0, None, ALU.mult)      # 4/rng
    s1 = pool.tile([128, n], F32, tag="normtmp")
    nc.vector.tensor_tensor(s1[:], mx_ap, negmn_ap, ALU.subtract)    # mx + mn
    nc.vector.tensor_tensor(s1[:], s1[:], rcp[:], ALU.mult)
    nc.vector.tensor_scalar(out_c, s1[:], -2.0, None, ALU.mult)


def build_bass(batch=BATCH, compile_=True, repeat=1, comm="full", pair=True):
    """Build the SPMD-8 Bass program. `batch` can be scaled down for sim."""
    b_local = batch // P_B
    n_blk = b_local // BBLK
    n_ic1, n_oc1 = IN1 // 128, HID // P_O // 128      # 8, 4
    n_ic2, n_oc2 = HID // 128, OUT // P_O // 128      # 16, 2
    bs_local = batch // N_CORES                       # stats slice per core

    nc = bacc.Bacc("TRN2", target_bir_lowering=False, debug=False,
                   num_devices=N_CORES)

    xh = nc.dram_tensor("xh", [IN1, b_local], F32, kind="ExternalInput").ap()
    xs = nc.dram_tensor("xs", [IN1, bs_local], F32, kind="ExternalInput").ap()
    w1 = nc.dram_tensor("w1", [128, n_ic1 * 8 * n_oc1 * 128], FP16,
                        kind="ExternalInput").ap()
    w2 = nc.dram_tensor("w2", [128, n_ic2 * 8 * n_oc2 * 128], FP16,
                        kind="ExternalInput").ap()
    bias1_d = nc.dram_tensor("bias1", [128, n_oc1], F32,
                             kind="ExternalInput").ap()
    bias2_d = nc.dram_tensor("bias2", [128, n_oc2], F32,
                             kind="ExternalInput").ap()
    y = nc.dram_tensor("y", [n_oc2 * 128, b_local], F32,
                       kind="ExternalOutput").ap()

    grp_og = [[bh * P_O + og for og in range(P_O)] for bh in range(P_B)]
    grp_all = [list(range(N_CORES))]

    with tile.TileContext(nc) as tc:
        with (
            tc.tile_pool(name="w", bufs=2) as wpool,
            tc.tile_pool(name="basis", bufs=4) as bpool,
            tc.tile_pool(name="xin", bufs=4) as xpool,
            tc.tile_pool(name="xstat", bufs=1) as xspool,
            tc.tile_pool(name="hst", bufs=4) as hpool,
            tc.tile_pool(name="small", bufs=1) as spool,
            tc.tile_pool(name="stmp", bufs=4) as stpool,
            tc.tile_pool(name="acc", bufs=8, space="PSUM") as ppool,
            tc.tile_pool(name="dstat", bufs=4, space="DRAM") as dspool,
            tc.tile_pool(name="dhin", bufs=max(n_blk, 3), space="DRAM") as dhin,
            tc.tile_pool(name="dhout", bufs=max(n_blk, 1), space="DRAM") as dhout,
        ):
            for _rep in range(repeat):
                # ---- biases ----
                neg2t = spool.tile([128, 1], F32)
                nc.vector.memset(neg2t[:], -2.0)
                b1sb = spool.tile([128, n_oc1], F32)
                nc.sync.dma_start(b1sb[:], bias1_d[:])
                b2sb = spool.tile([128, n_oc2], F32)
                nc.sync.dma_start(b2sb[:], bias2_d[:])

                # ======== L1 stats: per-feature min/max of x, all-reduced ======
                st1 = spool.tile([128, 2 * n_ic1], F32)
                for ic in range(n_ic1):
                    t = xspool.tile([128, bs_local], F32, tag="xstats")
                    nc.sync.dma_start(t[:], xs[ic * 128:(ic + 1) * 128, :])
                    nc.vector.tensor_reduce(st1[:, ic:ic + 1], t[:], AX.X, ALU.max)
                    mn = stpool.tile([128, 1], F32, tag="mn")
                    nc.vector.tensor_reduce(mn[:], t[:], AX.X, ALU.min)
                    nc.vector.tensor_scalar(st1[:, n_ic1 + ic:n_ic1 + ic + 1],
                                            mn[:], -1.0, None, ALU.mult)
                if comm != "none":
                    ar_in = dspool.tile([128, 2 * n_ic1], F32, tag="dstat")
                    ar_out = dspool.tile([128, 2 * n_ic1], F32, tag="dstat")
                    nc.sync.dma_start(ar_in[:], st1[:])
                    nc.gpsimd.collective_compute(
                        "AllReduce", ALU.max, replica_groups=grp_all,
                        ins=[ar_in.opt()], outs=[ar_out.opt()])
                    stg = spool.tile([128, 2 * n_ic1], F32)
                    nc.sync.dma_start(stg[:], ar_out[:])
                else:
                    stg = st1
                a1 = spool.tile([128, n_ic1], F32)
                c1 = spool.tile([128, n_ic1], F32)
                _emit_norm_scalars(nc, stpool, stg[:, 0:n_ic1], stg[:, n_ic1:],
                                   a1[:], c1[:], n_ic1)

                # ---- L1 weights ----
                wsb = wpool.tile([128, n_ic1 * 8 * n_oc1 * 128], FP16, tag="w")
                seg = 8 * n_oc1 * 128
                for ic in range(n_ic1):
                    nc.sync.dma_start(wsb[:, ic * seg:(ic + 1) * seg],
                                      w1[:, ic * seg:(ic + 1) * seg])

                # ---- L1 main loop ----
                hmax = spool.tile([128, n_oc1], F32)
                hmin = spool.tile([128, n_oc1], F32)
                hag_outs = []
                hag_ins = []
                if comm == "batched":
                    hag_in_full = dhin.tile([n_oc1 * 128, b_local], FP16,
                                            tag="hag_in_full")
                    hag_out_full = dhout.tile([HID, b_local], FP16,
                                              tag="hag_out_full")
                PAIR = 2 if (pair and n_blk % 2 == 0) else 1
                for bp in range(n_blk // PAIR):
                    blks = [bp * PAIR + j for j in range(PAIR)]
                    ps = [[ppool.tile([128, BBLK], F32, tag="acc",
                                      name=f"ps1_{_rep}_{blk}_{i}")
                           for i in range(n_oc1)] for blk in blks]
                    for ic in range(n_ic1):
                        bas = []
                        for blk in blks:
                            xt = xpool.tile([128, BBLK], F32, tag="x")
                            nc.sync.dma_start(
                                xt[:], xh[ic * 128:(ic + 1) * 128,
                                          blk * BBLK:(blk + 1) * BBLK])
                            bas.append(_emit_basis(
                                nc, bpool, xt[:], a1[:, ic:ic + 1],
                                c1[:, ic:ic + 1], BBLK, neg2t[:]))
                        for k in range(8):
                            for oc in range(n_oc1):
                                off = (((ic * 8 + k) * n_oc1 + oc) * 128)
                                for j in range(PAIR):
                                    nc.tensor.matmul(
                                        ps[j][oc][:], wsb[:, off:off + 128],
                                        bas[j][:, k * BBLK:(k + 1) * BBLK],
                                        start=(ic == 0 and k == 0),
                                        stop=(ic == n_ic1 - 1 and k == 7))
                    # epilogue: bias add, running h stats, stage + AllGather
                    for j, blk in enumerate(blks):
                        if comm == "batched":
                            hag_in = hag_in_full
                            hag_out = hag_out_full
                        else:
                            hag_in = dhin.tile([n_oc1 * 128, BBLK], FP16,
                                               tag="hag_in")
                            hag_out = dhout.tile([HID, BBLK], FP16,
                                                 tag="hag_out")
                        hag_outs.append(hag_out)
                        hag_ins.append(hag_in)
                        for oc in range(n_oc1):
                            hsb = hpool.tile([128, BBLK], FP16, tag="h")
                            nc.scalar.activation(hsb[:], ps[j][oc][:],
                                                 ACTF.Identity,
                                                 bias=b1sb[:, oc:oc + 1])
                            if blk == 0:
                                nc.vector.tensor_reduce(hmax[:, oc:oc + 1],
                                                        hsb[:], AX.X, ALU.max)
                                nc.vector.tensor_reduce(hmin[:, oc:oc + 1],
                                                        hsb[:], AX.X, ALU.min)
                            else:
                                tmx = stpool.tile([128, 1], F32, tag="tmx")
                                nc.vector.tensor_reduce(tmx[:], hsb[:], AX.X,
                                                        ALU.max)
                                nc.vector.tensor_tensor(hmax[:, oc:oc + 1],
                                                        hmax[:, oc:oc + 1],
                                                        tmx[:], ALU.max)
                                tmn = stpool.tile([128, 1], F32, tag="tmn")
                                nc.vector.tensor_reduce(tmn[:], hsb[:], AX.X,
                                                        ALU.min)
                                nc.vector.tensor_tensor(hmin[:, oc:oc + 1],
                                                        hmin[:, oc:oc + 1],
                                                        tmn[:], ALU.min)
                            if comm == "batched":
                                nc.sync.dma_start(
                                    hag_in[oc * 128:(oc + 1) * 128,
                                           blk * BBLK:(blk + 1) * BBLK],
                                    hsb[:])
                            else:
                                nc.sync.dma_start(
                                    hag_in[oc * 128:(oc + 1) * 128, :], hsb[:])
                        if comm == "full":
                            nc.gpsimd.collective_compute(
                                "AllGather", ALU.bypass, replica_groups=grp_og,
                                ins=[hag_in.opt()], outs=[hag_out.opt()])
                if comm == "batched":
                    nc.gpsimd.collective_compute(
                        "AllGather", ALU.bypass, replica_groups=grp_og,
                        ins=[hag_in_full.opt()], outs=[hag_out_full.opt()])

                # ======== L2 stats ========
                st2 = spool.tile([128, 2 * n_oc1], F32)
                nc.vector.tensor_copy(st2[:, 0:n_oc1], hmax[:])
                nc.vector.tensor_scalar(st2[:, n_oc1:], hmin[:], -1.0, None,
                                        ALU.mult)
                ag2_in = dspool.tile([128, 2 * n_oc1], F32, tag="dstat")
                ag2_out = dspool.tile([128 * N_CORES, 2 * n_oc1], F32,
                                      tag="dstat2")
                nc.sync.dma_start(ag2_in[:], st2[:])
                if comm != "none":
                    nc.gpsimd.collective_compute(
                        "AllGather", ALU.bypass, replica_groups=grp_all,
                        ins=[ag2_in.opt()], outs=[ag2_out.opt()])
                else:
                    nc.sync.dma_start(ag2_out[0:128, :], ag2_in[:])
                g2 = spool.tile([128, N_CORES * 2 * n_oc1], F32)
                nc.sync.dma_start(
                    g2[:], ag2_out[:].rearrange("(r p) c -> p r c", p=128))
                half = P_O * 2 * n_oc1
                m2 = spool.tile([128, half], F32)
                nc.vector.tensor_tensor(m2[:], g2[:, 0:half], g2[:, half:],
                                        ALU.max)
                # compact the strided og-blocked layout into [128, n_ic2]
                mxc = spool.tile([128, n_ic2], F32)
                nmc = spool.tile([128, n_ic2], F32)
                m2v = m2[:].rearrange("p (og c) -> p og c", og=P_O)
                nc.vector.tensor_copy(
                    mxc[:].rearrange("p (og c) -> p og c", og=P_O),
                    m2v[:, :, 0:n_oc1])
                nc.vector.tensor_copy(
                    nmc[:].rearrange("p (og c) -> p og c", og=P_O),
                    m2v[:, :, n_oc1:2 * n_oc1])
                a2 = spool.tile([128, n_ic2], F32)
                c2 = spool.tile([128, n_ic2], F32)
                _emit_norm_scalars(nc, stpool, mxc[:], nmc[:], a2[:], c2[:],
                                   n_ic2)

                # ---- L2 weights (reuses the w slot; waits for last w1 use) ----
                wsb2 = wpool.tile([128, n_ic2 * 8 * n_oc2 * 128], FP16, tag="w")
                seg2 = 8 * n_oc2 * 128
                for ic in range(n_ic2):
                    nc.sync.dma_start(wsb2[:, ic * seg2:(ic + 1) * seg2],
                                      w2[:, ic * seg2:(ic + 1) * seg2])

                # ---- L2 main loop ----
                for bp in range(n_blk // PAIR):
                    blks = [bp * PAIR + j for j in range(PAIR)]
                    ps = [[ppool.tile([128, BBLK], F32, tag="acc",
                                      name=f"ps2_{_rep}_{blk}_{i}")
                           for i in range(n_oc2)] for blk in blks]
                    for ic in range(n_ic2):
                        src_row = (ic * 128 if comm != "none"
                                   else (ic % n_oc1) * 128)
                        bas = []
                        for blk in blks:
                            ht = xpool.tile([128, BBLK], FP16, tag="x")
                            src_hag = (hag_outs[blk] if comm != "none"
                                       else hag_ins[blk])
                            if comm == "batched":
                                nc.sync.dma_start(
                                    ht[:], src_hag[src_row:src_row + 128,
                                                   blk * BBLK:(blk + 1) * BBLK])
                            else:
                                nc.sync.dma_start(
                                    ht[:], src_hag[src_row:src_row + 128, :])
                            bas.append(_emit_basis(
                                nc, bpool, ht[:], a2[:, ic:ic + 1],
                                c2[:, ic:ic + 1], BBLK, neg2t[:]))
                        for k in range(8):
                            for oc in range(n_oc2):
                                off = (((ic * 8 + k) * n_oc2 + oc) * 128)
                                for j in range(PAIR):
                                    nc.tensor.matmul(
                                        ps[j][oc][:], wsb2[:, off:off + 128],
                                        bas[j][:, k * BBLK:(k + 1) * BBLK],
                                        start=(ic == 0 and k == 0),
                                        stop=(ic == n_ic2 - 1 and k == 7))
                    for j, blk in enumerate(blks):
                        for oc in range(n_oc2):
                            ysb = hpool.tile([128, BBLK], F32, tag="h")
                            nc.scalar.activation(ysb[:], ps[j][oc][:],
                                                 ACTF.Identity,
                                                 bias=b2sb[:, oc:oc + 1])
                            nc.sync.dma_start(
                                y[oc * 128:(oc + 1) * 128,
                                  blk * BBLK:(blk + 1) * BBLK], ysb[:])

    if compile_:
        nc.compile()
    return nc


# --------------------------------------------------------------------------
# host wrapper
# --------------------------------------------------------------------------

def prepare_in_maps(x, coeffs1, coeffs2, batch=BATCH):
    xT = np.ascontiguousarray(x.T)                       # [IN1, batch]
    b_local = batch // P_B
    bs_local = batch // N_CORES

    p1, k1 = _to_product_basis(coeffs1)                  # [IN1, HID, 8]
    p2, k2 = _to_product_basis(coeffs2)
    bias1 = k1.sum(axis=0)                               # [HID]
    bias2 = k2.sum(axis=0)                               # [OUT]

    o1, o2 = HID // P_O, OUT // P_O
    in_maps = []
    packs = []
    for og in range(P_O):
        packs.append((
            _pack_weights(p1[:, og * o1:(og + 1) * o1, :], IN1 // 128,
                          o1 // 128),
            _pack_bias(bias1[og * o1:(og + 1) * o1]),
            _pack_weights(p2[:, og * o2:(og + 1) * o2, :], HID // 128,
                          o2 // 128),
            _pack_bias(bias2[og * o2:(og + 1) * o2]),
        ))
    for core in range(N_CORES):
        bh, og = divmod(core, P_O)
        w1p, b1p, w2p, b2p = packs[og]
        in_maps.append(dict(
            xh=np.ascontiguousarray(xT[:, bh * b_local:(bh + 1) * b_local]),
            xs=np.ascontiguousarray(
                xT[:, core * bs_local:(core + 1) * bs_local]),
            w1=w1p, w2=w2p, bias1=b1p, bias2=b2p,
        ))
    return in_maps


def assemble_output(results, batch=BATCH):
    b_local = batch // P_B
    o2 = OUT // P_O
    y = np.empty((batch, OUT), np.float32)
    for core in range(N_CORES):
        bh, og = divmod(core, P_O)
        y[bh * b_local:(bh + 1) * b_local, og * o2:(og + 1) * o2] = \
            results[core]["y"].T
    return y


def _disable_birsim():
    """Skip walrus's BIR-simulator pass (compile-time only; big speedup).
    Optionally (KAN_LDW_OPT=1) enable walrus's ldweights optimization."""
    import os
    import concourse.bass_utils as bu
    if getattr(bu, "_birsim_patched", False):
        return
    orig = bu.run_command
    ldw = os.environ.get("KAN_LDW_OPT") == "1"

    def patched(cmd, **kw):
        def fix(c):
            if not isinstance(c, str):
                return c
            c = c.replace("--enable-birsim=true", "--enable-birsim=false")
            if ldw:
                c = c.replace("--enable-ldw-opt=false", "--enable-ldw-opt=true")
            return c
        return orig([fix(c) for c in cmd], **kw)

    bu.run_command = patched
    bu._birsim_patched = True


_NC_CACHE = {}


def kernel(x, coeffs1, coeffs2):
    assert x.shape == (BATCH, IN1), x.shape
    _disable_birsim()
    if "nc" not in _NC_CACHE:
        _NC_CACHE["nc"] = build_bass(BATCH, comm="full")
    nc = _NC_CACHE["nc"]
    in_maps = prepare_in_maps(np.asarray(x, np.float32),
                              np.asarray(coeffs1, np.float32),
                              np.asarray(coeffs2, np.float32), BATCH)
    res = run_bass_kernel_spmd(nc, in_maps, core_ids=list(range(N_CORES)))
    return assemble_output(res.results, BATCH)

